# revision 31
# baseline (speedup 1.0000x reference)
"""Trainium2 Bass kernel for nn_Custom_trainer_79242146611896 (v4).

Data-parallel over N=16384 samples across 8 NeuronCores (2048/core).

Structure:
  - all matmuls/transposes in bf16 (1 cyc/row); x / output transposed in
    f32r mode (1.5 cyc/row, exact)
  - W_dec @ W_enc fused into WW [D,D] (built in the AllReduce#1 shadow):
    rec_latents = tanh(enc @ WW + (b_enc + b_dec @ W_enc))
  - rec diff built entirely inside PSUM: decodedT chain += b_dec (row
    matmul) -= outputT (neg-identity matmul); decoded never hits SBUF
  - per-chunk emission keeps the PE stream continuous (all transposes,
    then all matmul chains) so the PE p-state ramps to full clock
  - weights other than W_enc load lazily inside early chunks; pass A
    kicks AllReduce #1 (seg sums+counts) which hides under WW + mm3 +
    pass B; scalar partials go through a small AllGather (lower constant
    cost than AllReduce) hidden under means-prep + nsq + q-loop
  - Ln ops grouped: 2 activation-table swaps total
  - GpSimd/Pool only does iota + collectives (its ALU is far too slow)
"""

import numpy as np

import concourse.bass as bass
import concourse.mybir as mybir
import concourse.tile as tile
from concourse import bacc
from concourse.bass_utils import run_bass_kernel_spmd
from concourse.masks import make_identity

F32 = mybir.dt.float32
F32R = mybir.dt.float32r
BF16 = mybir.dt.bfloat16
I32 = mybir.dt.int32
AX = mybir.AxisListType
ALU = mybir.AluOpType
ACTF = mybir.ActivationFunctionType

P = 128
NCORES = 8
N_GLOBAL = 16384
T = 2048
D = 512
C = 50
KEPS = 1e-7


def build(nl=N_GLOBAL // NCORES, nc_chunk=256, n_global=None):
    n_global = n_global or NCORES * nl
    NT = T // P          # 16 T-tiles
    ND = D // P          # 4 D-tiles
    NN = nl // P         # 16 sample-tiles per core
    NC = nc_chunk        # samples per chunk (256)
    NCH = nl // NC       # 8 chunks
    NSUB = NC // P       # 2 sample-tiles per chunk
    RSQD = 1.0 / float(np.sqrt(D))   # nsq/msq come out pre-divided by D

    nc = bacc.Bacc("TRN2", target_bir_lowering=False, debug=False, num_devices=NCORES)

    x_d = nc.dram_tensor("x", [nl, T], F32R, kind="ExternalInput")
    o_d = nc.dram_tensor("output", [nl, T], F32R, kind="ExternalInput")
    cl_d = nc.dram_tensor("cat_labels", [nl, C], F32, kind="ExternalInput")
    lab_d = nc.dram_tensor("labels", [nl], I32, kind="ExternalInput")
    wenc_d = nc.dram_tensor("W_enc", [T, D], F32, kind="ExternalInput")
    benc_d = nc.dram_tensor("b_enc", [D], F32, kind="ExternalInput")
    wdec_d = nc.dram_tensor("W_dec", [D, T], F32, kind="ExternalInput")
    bdec_d = nc.dram_tensor("b_dec", [T], F32, kind="ExternalInput")
    wcls_d = nc.dram_tensor("W_cls", [D, C], F32, kind="ExternalInput")
    bcls_d = nc.dram_tensor("b_cls", [C], F32, kind="ExternalInput")
    out_d = nc.dram_tensor("out", [nl], F32, kind="ExternalOutput")

    from contextlib import ExitStack

    with tile.TileContext(nc) as tc:
        with ExitStack() as ctx:
            ent = ctx.enter_context
            constp = ent(tc.tile_pool(name="const", bufs=1))
            wts = ent(tc.tile_pool(name="wts", bufs=1))
            encp = ent(tc.tile_pool(name="enc", bufs=1))
            accp = ent(tc.tile_pool(name="acc", bufs=1))
            junkp = ent(tc.tile_pool(name="junk", bufs=2))
            dp = ent(tc.tile_pool(name="dram", bufs=1, space="DRAM"))

            # ---------------- constants ----------------
            ident_f32 = constp.tile([P, P], F32)
            make_identity(nc, ident_f32)
            ident_bf = constp.tile([P, P], BF16)
            nc.vector.tensor_copy(ident_bf[:], ident_f32[:])
            ident_fr = constp.tile([P, P], F32R)
            nc.vector.tensor_copy(ident_fr[:], ident_f32[:])
            nident_bf = constp.tile([P, P], BF16)
            nc.vector.tensor_scalar(
                out=nident_bf[:], in0=ident_f32[:], scalar1=-1.0, scalar2=None,
                op0=ALU.mult,
            )

            ones_col = constp.tile([P, 1], F32)
            nc.any.memset(ones_col[:], 1.0)
            ones_col_bf = constp.tile([P, 1], BF16)
            nc.any.memset(ones_col_bf[:], 1.0)
            ones_k1f = constp.tile([1, P], F32)
            nc.any.memset(ones_k1f[:], 1.0)
            ones_k1b = constp.tile([1, P], BF16)
            nc.any.memset(ones_k1b[:], 1.0)
            ones_row2 = constp.tile([1, NC], BF16)
            nc.any.memset(ones_row2[:], 1.0)

            iot = constp.tile([P, C], I32)
            nc.gpsimd.iota(iot[:], [[1, C]], channel_multiplier=0)
            iotaf = constp.tile([P, C], F32)
            nc.vector.tensor_copy(iotaf[:], iot[:])

            encT = [encp.tile([P, nl], BF16, name=f"encT{k}", tag=f"encT{k}")
                    for k in range(ND)]
            en_t = [encp.tile([P, D], BF16, name=f"en{i}", tag=f"en{i}")
                    for i in range(NN)]

            nsq_strip = accp.tile([P, NN], F32)
            rec_strip = accp.tile([P, NCH * 8], F32)
            lat_strip = accp.tile([P, NN], F32)
            cat_strip = accp.tile([P, NN], F32)

            # ======== ramp + PASS A ========
            with tc.tile_pool(name="stg", bufs=1) as stg, \
                 tc.tile_pool(name="wdt", bufs=1) as wdtp, \
                 tc.tile_pool(name="xr", bufs=2) as xrp, \
                 tc.tile_pool(name="xt", bufs=2) as xtp, \
                 tc.tile_pool(name="ps_xt", bufs=2, space="PSUM") as ps_xt, \
                 tc.tile_pool(name="ps_mm1", bufs=2, space="PSUM") as ps_mm1, \
                 tc.tile_pool(name="ps_en", bufs=1, space="PSUM") as ps_en, \
                 tc.tile_pool(name="ps_seg", bufs=1, space="PSUM") as ps_seg, \
                 tc.tile_pool(name="ps_misc", bufs=2, space="PSUM") as ps_miscp:

                def load_x(c):
                    rs = []
                    for s in range(NSUB):
                        r_ = xrp.tile([P, T], F32R, tag=f"xr{s}")
                        nc.sync.dma_start(
                            r_[:], x_d[c * NC + s * P : c * NC + (s + 1) * P, :]
                        )
                        rs.append(r_)
                    return rs

                xrow_c = load_x(0)

                # W_enc first: mm1 needs it
                wenc_r = wenc_d.ap().rearrange("(a p) d -> a p d", p=P)
                wenc = []
                for t in range(NT):
                    s_ = stg.tile([P, D], F32, tag="stgd")
                    nc.sync.dma_start(s_[:], wenc_r[t])
                    w_ = wts.tile([P, D], BF16, tag=f"wenc{t}")
                    if t % 2:
                        nc.scalar.activation(w_[:], s_[:], ACTF.Copy)
                    else:
                        nc.vector.tensor_copy(w_[:], s_[:])
                    wenc.append(w_)
                benc_r = benc_d.ap().rearrange("(a p) -> a p", p=P)
                bencT = []
                for k in range(ND):
                    b_ = wts.tile([P, 1], F32, tag=f"bencT{k}")
                    nc.sync.dma_start(b_[:], benc_r[k].rearrange("(p o) -> p o", o=1))
                    bencT.append(b_)

                # lazy-load state filled in by chunk bodies
                wdec = []
                wcls = []
                bdecT = []
                lazy = {}

                def lazy_chunk_loads(c):
                    if c == 0:
                        wdec_r = wdec_d.ap().rearrange("(a p) t -> a p t", p=P)
                        for k in range(ND):
                            s_ = stg.tile([P, T], F32, tag="stgt")
                            nc.sync.dma_start(s_[:], wdec_r[k])
                            w_ = wts.tile([P, T], BF16, tag=f"wdec{k}")
                            if k % 2:
                                nc.scalar.activation(w_[:], s_[:], ACTF.Copy)
                            else:
                                nc.vector.tensor_copy(w_[:], s_[:])
                            wdec.append(w_)
                    elif c == 1:
                        labi_all = accp.tile([P, NN], I32)
                        for i in range(NN):
                            nc.sync.dma_start(
                                labi_all[:, i : i + 1],
                                lab_d[i * P : (i + 1) * P].rearrange(
                                    "(p o) -> p o", o=1
                                ),
                            )
                        labf_all = accp.tile([P, NN], F32)
                        nc.vector.tensor_copy(labf_all[:], labi_all[:])
                        oh_all = accp.tile([P, NN * C], F32)
                        oh_bf = accp.tile([P, NN * C], BF16)
                        for i in range(NN):
                            nc.vector.tensor_scalar(
                                out=oh_all[:, i * C : (i + 1) * C], in0=iotaf[:],
                                scalar1=labf_all[:, i : i + 1], scalar2=None,
                                op0=ALU.is_equal,
                            )
                        nc.vector.tensor_copy(oh_bf[:], oh_all[:])
                        lazy["oh_all"] = oh_all
                        lazy["oh_bf"] = oh_bf
                    elif c == 2:
                        catl_all = accp.tile([P, NN * C], F32)
                        nc.sync.dma_start(
                            catl_all[:], cl_d.ap().rearrange("(a p) c -> p a c", p=P)
                        )
                        lazy["catl_all"] = catl_all
                    elif c == 3:
                        wcls_r = wcls_d.ap().rearrange("(a p) c -> a p c", p=P)
                        for k in range(ND):
                            s_ = stg.tile([P, C], F32, tag="stgc")
                            nc.sync.dma_start(s_[:], wcls_r[k])
                            w_ = wts.tile([P, C], BF16, tag=f"wcls{k}")
                            nc.vector.tensor_copy(w_[:], s_[:])
                            wcls.append(w_)
                        bcls_row_f = stg.tile([1, C], F32, tag="stgcr")
                        nc.sync.dma_start(
                            bcls_row_f[:], bcls_d.ap().rearrange("(o c) -> o c", o=1)
                        )
                        bcls_row = wts.tile([1, C], BF16)
                        nc.vector.tensor_copy(bcls_row[:], bcls_row_f[:])
                        lazy["bcls_row"] = bcls_row
                    elif c == 4:
                        benc_row_f = stg.tile([1, D], F32, tag="stgbr")
                        nc.sync.dma_start(
                            benc_row_f[:],
                            benc_d.ap().rearrange("(o d) -> o d", o=1),
                        )
                        lazy["benc_row_f"] = benc_row_f
                        bdec_row_f = stg.tile([1, T], F32, tag="stgdr")
                        nc.sync.dma_start(
                            bdec_row_f[:],
                            bdec_d.ap().rearrange("(o t) -> o t", o=1),
                        )
                        bdec_row = wts.tile([1, T], BF16)
                        nc.vector.tensor_copy(bdec_row[:], bdec_row_f[:])
                        lazy["bdec_row"] = bdec_row
                    elif c == 5:
                        bdec_r = bdec_d.ap().rearrange("(a p) -> a p", p=P)
                        for t in range(NT):
                            s_ = stg.tile([P, 1], F32, tag="stgb")
                            nc.sync.dma_start(
                                s_[:], bdec_r[t].rearrange("(p o) -> p o", o=1)
                            )
                            b_ = wts.tile([P, 1], BF16, tag=f"bdecT{t}")
                            nc.vector.tensor_copy(b_[:], s_[:])
                            bdecT.append(b_)

                # ---- PASS A chunks ----
                for c in range(NCH):
                    base = c * NC
                    xr = xrow_c
                    if c + 1 < NCH:
                        xrow_nxt = load_x(c + 1)
                    lazy_chunk_loads(c)

                    # (a) all x transposes of the chunk (PE stream), copies chase
                    xt_all = xtp.tile([P, NT * NC], BF16, tag="xtall")
                    ncopy = 0
                    for g in range(NT // 2):
                        tpw = ps_xt.tile([P, 2 * NC], F32R, tag="psxt")
                        for j in range(2):
                            t = 2 * g + j
                            for s in range(NSUB):
                                nc.tensor.transpose(
                                    tpw[:, j * NC + s * P : j * NC + (s + 1) * P],
                                    xr[s][:, t * P : (t + 1) * P],
                                    ident_fr[:],
                                )
                        if ncopy % 2:
                            nc.scalar.activation(
                                xt_all[:, g * 2 * NC : (g + 1) * 2 * NC], tpw[:],
                                ACTF.Copy,
                            )
                        else:
                            nc.vector.tensor_copy(
                                xt_all[:, g * 2 * NC : (g + 1) * 2 * NC], tpw[:]
                            )
                        ncopy += 1

                    # (b) mm1 chains
                    for k in range(ND):
                        ps_t = ps_mm1.tile([P, D], F32, tag="psmm1")
                        ps = ps_t[:, 0:NC]
                        for t in range(NT):
                            nc.tensor.matmul(
                                ps, wenc[t][:, k * P : (k + 1) * P],
                                xt_all[:, t * NC : (t + 1) * NC],
                                start=(t == 0), stop=(t == NT - 1),
                            )
                        nc.scalar.activation(
                            encT[k][:, base : base + NC], ps, ACTF.Tanh,
                            bias=bencT[k][:],
                        )

                    # (c) en natural
                    for s in range(NSUB):
                        i = c * NSUB + s
                        tpe = ps_en.tile([P, D], BF16, tag="psen")
                        for k in range(ND):
                            nc.tensor.transpose(
                                tpe[:, k * P : (k + 1) * P],
                                encT[k][:, base + s * P : base + (s + 1) * P],
                                ident_bf[:],
                            )
                        if s % 2:
                            nc.vector.tensor_copy(en_t[i][:], tpe[:])
                        else:
                            nc.scalar.activation(en_t[i][:], tpe[:], ACTF.Copy)
                    xrow_c = xrow_nxt if c + 1 < NCH else None

                oh_all = lazy["oh_all"]
                oh_bf = lazy["oh_bf"]
                catl_all = lazy["catl_all"]
                bcls_row = lazy["bcls_row"]
                bdec_row = lazy["bdec_row"]

                # ---- segment sums + counts (chained PSUM accumulation) ----
                seg_ps = ps_seg.tile([C, D], F32)
                cnt_t = ps_miscp.tile([P, D], F32, tag="psmisc")
                cnt_ps = cnt_t[0:C, 0:1]
                for i in range(NN):
                    nc.tensor.matmul(
                        seg_ps[:], oh_bf[:, i * C : (i + 1) * C], en_t[i][:],
                        start=(i == 0), stop=(i == NN - 1),
                    )
                for i in range(NN):
                    nc.tensor.matmul(
                        cnt_ps, oh_bf[:, i * C : (i + 1) * C], ones_col_bf[:],
                        start=(i == 0), stop=(i == NN - 1),
                    )
                seg_sb = accp.tile([C, D], F32)
                nc.scalar.activation(seg_sb[:], seg_ps[:], ACTF.Copy)
                counts_sb = accp.tile([C, 1], F32)
                nc.scalar.activation(counts_sb[:], cnt_ps, ACTF.Copy)

                # AllReduce #1: [C, D+4] (seg sums + counts), kicked early
                bounce_in = dp.tile([C, D + 4], F32)
                bounce_out = dp.tile([C, D + 4], F32)
                zr3 = accp.tile([C, 3], F32)
                nc.any.memset(zr3[:], 0.0)
                nc.sync.dma_start(bounce_in[:, 0:D], seg_sb[:])
                nc.sync.dma_start(bounce_in[:, D : D + 1], counts_sb[:])
                nc.sync.dma_start(bounce_in[:, D + 1 : D + 4], zr3[:])
                nc.gpsimd.collective_compute(
                    "AllReduce",
                    ALU.add,
                    replica_groups=[list(range(NCORES))],
                    ins=[bounce_in[:].opt()],
                    outs=[bounce_out[:].opt()],
                )

                # ---- WW = W_dec @ W_enc, cr = b_enc + b_dec @ W_enc ----
                # (PE work hiding under AllReduce #1)
                wdecT = []
                for t in range(NT):
                    tpw = ps_en.tile([P, D], BF16, tag="psen")
                    for k in range(ND):
                        nc.tensor.transpose(
                            tpw[:, k * P : (k + 1) * P],
                            wdec[k][:, t * P : (t + 1) * P], ident_bf[:],
                        )
                    w_ = wdtp.tile([P, D], BF16, tag=f"wdecT{t}")
                    if t % 2:
                        nc.scalar.activation(w_[:], tpw[:], ACTF.Copy)
                    else:
                        nc.vector.tensor_copy(w_[:], tpw[:])
                    wdecT.append(w_)
                WW = []
                for g1 in range(ND):
                    psw = ps_mm1.tile([P, D], F32, tag="psmm1")
                    for t in range(NT):
                        nc.tensor.matmul(
                            psw[:], wdecT[t][:, g1 * P : (g1 + 1) * P], wenc[t][:],
                            start=(t == 0), stop=(t == NT - 1),
                        )
                    w_ = wts.tile([P, D], BF16, tag=f"WW{g1}")
                    if g1 % 2:
                        nc.scalar.activation(w_[:], psw[:], ACTF.Copy)
                    else:
                        nc.vector.tensor_copy(w_[:], psw[:])
                    WW.append(w_)
                pscr_t = ps_mm1.tile([P, D], F32, tag="psmm1")
                pscr = pscr_t[0:1, :]
                for t in range(NT):
                    nc.tensor.matmul(
                        pscr, bdecT[t][:], wenc[t][:],
                        start=(t == 0), stop=(t == NT - 1),
                    )
                crf = stg.tile([1, D], F32, tag="stgbr2")
                nc.vector.tensor_tensor(crf[:], pscr, lazy["benc_row_f"][:], ALU.add)
                cr_row = wts.tile([1, D], BF16)
                nc.vector.tensor_copy(cr_row[:], crf[:])

                # ---- mm3 block: logits/softmax/CCE (overlaps AllReduce #1) ----
                expt_all = accp.tile([P, NN * C], F32)
                sume_all = accp.tile([P, NN], F32)
                for i in range(NN):
                    ps3_t = ps_miscp.tile([P, D], F32, tag="psmisc")
                    ps3 = ps3_t[:, 0:C]
                    for k in range(ND):
                        nc.tensor.matmul(
                            ps3, encT[k][:, i * P : (i + 1) * P], wcls[k][:],
                            start=(k == 0), stop=False,
                        )
                    nc.tensor.matmul(
                        ps3, ones_k1b[:], bcls_row[:], start=False, stop=True
                    )
                    nc.scalar.activation(
                        expt_all[:, i * C : (i + 1) * C], ps3, ACTF.Exp,
                        accum_out=sume_all[:, i : i + 1],
                    )
                rcp_all = accp.tile([P, NN], F32)
                nc.vector.reciprocal(rcp_all[:], sume_all[:])

                rs_all = accp.tile([P, NN], F32)
                nc.vector.tensor_reduce(
                    rs_all[:], catl_all[:].rearrange("p (i c) -> p i c", c=C),
                    AX.X, ALU.add,
                )
                rr_all = accp.tile([P, NN], F32)
                nc.vector.reciprocal(rr_all[:], rs_all[:])
                yp_all = accp.tile([P, NN * C], F32)
                for i in range(NN):
                    nc.vector.tensor_scalar(
                        out=yp_all[:, i * C : (i + 1) * C],
                        in0=catl_all[:, i * C : (i + 1) * C],
                        scalar1=rr_all[:, i : i + 1], scalar2=None, op0=ALU.mult,
                    )
                yc_all = accp.tile([P, NN * C], F32)
                nc.vector.tensor_scalar(
                    out=yc_all[:], in0=yp_all[:],
                    scalar1=KEPS, scalar2=1.0 - KEPS, op0=ALU.max, op1=ALU.min,
                )
                lg_all = accp.tile([P, NN * C], F32)
                nc.scalar.activation(lg_all[:], yc_all[:], ACTF.Ln)
                pr_all = accp.tile([P, NN * C], F32)
                nc.vector.tensor_tensor(pr_all[:], expt_all[:], lg_all[:], ALU.mult)
                t1_all = accp.tile([P, NN], F32)
                nc.vector.tensor_reduce(
                    t1_all[:], pr_all[:].rearrange("p (i c) -> p i c", c=C),
                    AX.X, ALU.add,
                )
                nc.vector.tensor_tensor(cat_strip[:], t1_all[:], rcp_all[:], ALU.mult)

            # ======== PASS B ========
            with tc.tile_pool(name="ps_ot", bufs=2, space="PSUM") as ps_ot, \
                 tc.tile_pool(name="ps_dd", bufs=2, space="PSUM") as ps_dd, \
                 tc.tile_pool(name="ps_m4", bufs=2, space="PSUM") as ps_m4, \
                 tc.tile_pool(name="orow", bufs=2) as orowp, \
                 tc.tile_pool(name="ots", bufs=9) as otsp, \
                 tc.tile_pool(name="lt", bufs=2) as ltp, \
                 tc.tile_pool(name="d2p", bufs=2) as d2p:

                def load_o(c):
                    rs = []
                    for s in range(NSUB):
                        r_ = orowp.tile([P, T], F32R, tag=f"or{s}")
                        nc.sync.dma_start(
                            r_[:], o_d[c * NC + s * P : c * NC + (s + 1) * P, :]
                        )
                        rs.append(r_)
                    return rs

                orow_c = load_o(0)
                for c in range(NCH):
                    base = c * NC
                    orow = orow_c
                    if c + 1 < NCH:
                        orow_nxt = load_o(c + 1)

                    # (a) all output transposes of the chunk; copies chase
                    ots = []
                    for g in range(NT // 2):
                        tpo = ps_ot.tile([P, 2 * NC], F32R, tag="psot")
                        for j in range(2):
                            t = 2 * g + j
                            for s in range(NSUB):
                                nc.tensor.transpose(
                                    tpo[:, j * NC + s * P : j * NC + (s + 1) * P],
                                    orow[s][:, t * P : (t + 1) * P],
                                    ident_fr[:],
                                )
                        ot = otsp.tile([P, 2 * NC], BF16, tag="ot")
                        if g % 2:
                            nc.scalar.activation(ot[:], tpo[:], ACTF.Copy)
                        else:
                            nc.vector.tensor_copy(ot[:], tpo[:])
                        ots.append(ot)

                    # (b) dd chains: decodedT + b_dec - outputT in PSUM
                    for g in range(NT // 2):
                        dd = ps_dd.tile([P, 2 * NC], F32, tag="psdd")
                        for j in range(2):
                            t = 2 * g + j
                            sl = slice(j * NC, (j + 1) * NC)
                            for k in range(ND):
                                nc.tensor.matmul(
                                    dd[:, sl], wdec[k][:, t * P : (t + 1) * P],
                                    encT[k][:, base : base + NC],
                                    start=(k == 0), stop=False,
                                )
                            nc.tensor.matmul(
                                dd[:, sl], bdec_row[:, t * P : (t + 1) * P],
                                ones_row2[:], start=False, stop=False,
                            )
                            nc.tensor.matmul(
                                dd[:, sl], nident_bf[:], ots[g][:, sl],
                                start=False, stop=True,
                            )
                        col = c * 8 + g
                        if g % 2:
                            jb = junkp.tile([P, 2 * NC], BF16, tag="junkb")
                            nc.scalar.activation(
                                jb[:], dd[:], ACTF.Abs,
                                accum_out=rec_strip[:, col : col + 1],
                            )
                        else:
                            nc.vector.tensor_reduce(
                                rec_strip[:, col : col + 1], dd[:],
                                AX.X, ALU.add, apply_absolute_value=True,
                            )

                    # (c) fused rec_latents; lat pinball
                    for s in range(NSUB):
                        i = c * NSUB + s
                        ps4 = ps_m4.tile([P, D], F32, tag="psm4")
                        for k in range(ND):
                            nc.tensor.matmul(
                                ps4[:],
                                encT[k][:, base + s * P : base + (s + 1) * P],
                                WW[k][:], start=(k == 0), stop=False,
                            )
                        nc.tensor.matmul(
                            ps4[:], ones_k1b[:], cr_row[:], start=False, stop=True
                        )
                        lt = ltp.tile([P, D], BF16, tag="lt")
                        nc.scalar.activation(lt[:], ps4[:], ACTF.Tanh)
                        d2 = d2p.tile([P, D], BF16, tag="d2")
                        nc.vector.tensor_tensor(d2[:], lt[:], en_t[i][:], ALU.subtract)
                        nc.vector.tensor_reduce(
                            lat_strip[:, i : i + 1], d2[:], AX.X, ALU.add,
                            apply_absolute_value=True,
                        )
                    orow_c = orow_nxt if c + 1 < NCH else None

            # ======== tail ========
            with tc.tile_pool(name="ps_p2", bufs=2, space="PSUM") as ps_p2, \
                 tc.tile_pool(name="ps_q", bufs=2, space="PSUM") as ps_q, \
                 tc.tile_pool(name="p2s", bufs=2) as p2s:

                # scalar partials -> AllGather (lower constant cost), sum locally
                pk = accp.tile([P, 3], F32)
                nc.vector.tensor_reduce(pk[:, 0:1], rec_strip[:], AX.X, ALU.add)
                nc.vector.tensor_reduce(pk[:, 1:2], lat_strip[:], AX.X, ALU.add)
                nc.vector.tensor_reduce(pk[:, 2:3], cat_strip[:], AX.X, ALU.add)
                scps_t = ps_p2.tile([P, D], F32, tag="psp2")
                scps = scps_t[0:1, 0:3]
                nc.tensor.matmul(scps, ones_col[:], pk[:], start=True, stop=True)
                sc_row = accp.tile([1, 3], F32)
                nc.scalar.activation(sc_row[:], scps, ACTF.Copy)

                b2_in = dp.tile([1, 4], F32)
                b2_out = dp.tile([NCORES, 4], F32)
                zr8 = accp.tile([1, 4], F32)
                nc.any.memset(zr8[:], 0.0)
                nc.sync.dma_start(b2_in[:], zr8[:])
                nc.sync.dma_start(b2_in[:, 0:3], sc_row[:])
                nc.gpsimd.collective_compute(
                    "AllGather",
                    ALU.bypass,
                    replica_groups=[list(range(NCORES))],
                    ins=[b2_in[:].opt()],
                    outs=[b2_out[:].opt()],
                )

                # means prep (AllReduce #1 result)
                sums_g = accp.tile([C, D], F32)
                nc.sync.dma_start(sums_g[:], bounce_out[:, 0:D])
                counts_g = accp.tile([C, 1], F32)
                nc.sync.dma_start(counts_g[:], bounce_out[:, D : D + 1])

                cmax = accp.tile([C, 1], F32)
                nc.vector.tensor_scalar(
                    out=cmax[:], in0=counts_g[:], scalar1=1.0, scalar2=None,
                    op0=ALU.max,
                )
                crcp = accp.tile([C, 1], F32)
                nc.vector.reciprocal(crcp[:], cmax[:])
                means = accp.tile([C, D], F32)
                nc.vector.tensor_scalar(
                    out=means[:], in0=sums_g[:], scalar1=crcp[:], scalar2=None,
                    op0=ALU.mult,
                )
                msq_col = accp.tile([C, 1], F32)
                jm = junkp.tile([C, D], BF16, tag="junkm")
                nc.scalar.activation(
                    jm[:], means[:], ACTF.Square, scale=RSQD, accum_out=msq_col[:]
                )

                meansT = []
                for k in range(ND):
                    tpm_t = ps_p2.tile([P, D], F32, tag="psp2")
                    tpm = tpm_t[:, 0:C]
                    nc.tensor.transpose(
                        tpm, means[:, k * P : (k + 1) * P], ident_f32[:C, :C]
                    )
                    mt = p2s.tile([P, C], BF16, tag=f"mT{k}")
                    nc.vector.tensor_copy(mt[:], tpm)
                    meansT.append(mt)

                tpq_t = ps_p2.tile([P, D], F32, tag="psp2")
                tpq = tpq_t[0:1, 0:C]
                nc.tensor.transpose(tpq, msq_col[:], ident_f32[:C, :C])
                msq_row4 = p2s.tile([1, 4 * C], F32, tag="msqr")
                for j in range(4):
                    nc.vector.tensor_copy(msq_row4[:, j * C : (j + 1) * C], tpq)
                psb4_t = ps_p2.tile([P, D], F32, tag="psp2")
                psb4 = psb4_t[:, 0 : 4 * C]
                nc.tensor.matmul(
                    psb4, ones_k1f[:], msq_row4[:], start=True, stop=True
                )
                msq_b4 = p2s.tile([P, 4 * C], F32, tag="msqb")
                nc.scalar.activation(msq_b4[:], psb4, ACTF.Copy)

                # nsq block (deferred here to overlap the AllGather)
                for i in range(NN):
                    jn = junkp.tile([P, D], BF16, tag="junk")
                    nc.scalar.activation(
                        jn[:], en_t[i][:], ACTF.Square, scale=RSQD,
                        accum_out=nsq_strip[:, i : i + 1],
                    )

                # phase 2 q-loop (overlaps the AllGather)
                gq_strip = accp.tile([P, NN], F32)
                for g in range(NN // 4):
                    psq = ps_q.tile([P, 4 * C], F32, tag="psq")
                    for j in range(4):
                        i = 4 * g + j
                        for k in range(ND):
                            nc.tensor.matmul(
                                psq[:, j * C : (j + 1) * C],
                                encT[k][:, i * P : (i + 1) * P], meansT[k][:],
                                start=(k == 0), stop=(k == ND - 1),
                            )
                    qt = p2s.tile([P, 4 * C], F32, tag="qt")
                    nc.vector.scalar_tensor_tensor(
                        out=qt[:], in0=psq[:], scalar=-2.0 * RSQD * RSQD,
                        in1=msq_b4[:], op0=ALU.mult, op1=ALU.add,
                    )
                    j4 = p2s.tile([P, 4 * C], F32, tag="j4")
                    nc.vector.tensor_tensor(
                        j4[:], qt[:], oh_all[:, g * 4 * C : (g + 1) * 4 * C], ALU.mult
                    )
                    nc.vector.tensor_reduce(
                        gq_strip[:, g * 4 : (g + 1) * 4],
                        j4[:].rearrange("p (i c) -> p i c", c=C),
                        AX.X, ALU.add,
                    )
                t2_strip = accp.tile([P, NN], F32)
                nc.vector.tensor_tensor(t2_strip[:], gq_strip[:], nsq_strip[:], ALU.add)

                # final: sum gathered partials, + global scalar, output
                sc_g = accp.tile([NCORES, 4], F32)
                nc.sync.dma_start(sc_g[:], b2_out[:])
                coef = accp.tile([1, 3], F32)
                nc.any.memset(coef[:, 0:1], 0.9 / (float(n_global) * T))
                nc.any.memset(coef[:, 1:2], 0.9 / (float(n_global) * D))
                nc.any.memset(coef[:, 2:3], -1.0 / float(n_global))
                psg_t = ps_p2.tile([P, D], F32, tag="psp2")
                nc.tensor.matmul(
                    psg_t[0:1, 0:4], ones_col[0:NCORES, :], sc_g[:],
                    start=True, stop=True,
                )
                sprod = accp.tile([1, 3], F32)
                nc.vector.tensor_tensor(sprod[:], psg_t[0:1, 0:3], coef[:], ALU.mult)
                stot = accp.tile([1, 1], F32)
                nc.vector.tensor_reduce(stot[:], sprod[:], AX.X, ALU.add)
                psS = ps_p2.tile([P, D], F32, tag="psp2")
                nc.tensor.matmul(
                    psS[:, 0:1], ones_k1f[:], stot[:], start=True, stop=True
                )
                s_col = accp.tile([P, 1], F32)
                nc.scalar.activation(s_col[:], psS[:, 0:1], ACTF.Copy)

                out_strip = accp.tile([P, NN], F32)
                nc.vector.tensor_scalar(
                    out=out_strip[:], in0=t2_strip[:],
                    scalar1=s_col[:], scalar2=None, op0=ALU.add,
                )
                psT_t = ps_p2.tile([P, D], F32, tag="psp2")
                psT = psT_t[0:NN, 0:P]
                nc.tensor.transpose(psT, out_strip[:], ident_f32[:])
                outT = accp.tile([NN, P], F32)
                nc.scalar.activation(outT[:], psT, ACTF.Copy)
                nc.sync.dma_start(
                    out_d.ap().rearrange("(a p) -> a p", p=P), outT[:]
                )

    nc.compile()
    return nc


_CACHE = {}


def _get_nc():
    if "nc" not in _CACHE:
        _CACHE["nc"] = build()
    return _CACHE["nc"]


def kernel(**inputs):
    nc = _get_nc()
    nl = N_GLOBAL // NCORES
    shard_names = ["x", "output", "cat_labels", "labels"]
    full_names = ["W_enc", "b_enc", "W_dec", "b_dec", "W_cls", "b_cls"]
    in_maps = []
    for i in range(NCORES):
        m = {}
        for k in shard_names:
            m[k] = np.ascontiguousarray(inputs[k][i * nl : (i + 1) * nl])
        for k in full_names:
            m[k] = np.ascontiguousarray(inputs[k])
        in_maps.append(m)
    res = run_bass_kernel_spmd(nc, in_maps, list(range(NCORES))).results
    return np.concatenate([res[i]["out"] for i in range(NCORES)]).astype(np.float32)
